# revision 1
# baseline (speedup 1.0000x reference)
"""Distributed GNN message-passing kernel for 8 TRN2 NeuronCores.

Strategy (per sharding hint): nodes sharded across 8 cores (12500 real +
44 pad = 12544 rows/core). Edges partitioned by destination core, grouped
into (dst-tile of 128 nodes) x (source bucket of 25088 table rows) cells.
Per layer:
  A: support = h_norm @ W computed locally (feature-major stationary).
  B: support replicated to all cores via 16 chunked AllGathers into a
     4-bucket interleaved table (chunk-major row order so each collective
     chunk lands contiguously and buckets complete in order).
  C: per (bucket, tile): dma_gather source rows (int16 idx into the
     25088-row bucket), build a 0/1 selection matrix (dst one-hot via
     is_equal against an iota row), scale gathered rows by edge weight,
     and accumulate h_new[tile] = sel.T @ (w * G) on the TensorEngine.
  D: BatchNorm stats (column sums via ones-matmul, AllReduce, scale/shift).
Normalization + leaky-relu is folded into the next layer's transpose step
(per-feature affine becomes per-partition after PE transpose).
"""

import os
import sys
import numpy as np

sys.path.insert(0, "/opt/trn_rl_repo")

from concourse import bass, bacc, mybir, tile, library_config  # noqa: E402
from concourse.bass_utils import run_bass_kernel_spmd  # noqa: E402

F32 = mybir.dt.float32
I16 = mybir.dt.int16
AF = mybir.ActivationFunctionType
OP = mybir.AluOpType
AX = mybir.AxisListType

NCORES = 8
N = 100000
NE = 3200000
D = 256
NCLASS = 40
NL = 4
ALPHA = 0.2
BN_EPS = 1e-5
LN_EPS = 1e-5
SLOPE = 0.3

P = 128


def _set_scale(sr, n, nchunk=16, nb=4):
    """Set shard geometry (globals). Default is the real problem size;
    smaller values are used by the simulator harness for debugging."""
    global SR, SP, NT, NPAD, NCHUNK, CS, CT, NB, BK, N
    N = n
    SR = sr
    SP = ((sr + P - 1) // P) * P
    NT = SP // P
    NPAD = SP * NCORES
    NCHUNK = nchunk
    assert SP % NCHUNK == 0
    CS = SP // NCHUNK
    CT = CS * NCORES
    NB = nb
    assert NPAD % NB == 0 and (NPAD // NB) % CT == 0
    BK = NPAD // NB
    assert BK <= 32768


_set_scale(12500, 100000)

_CACHE = {}


def _preprocess(x, edge_src, edge_dst, edge_w):
    """Host-side: shard x, map sources to interleaved table ids, group edges
    into (core, dst-tile, bucket) cells, and build gather/selection arrays."""
    es = np.asarray(edge_src, np.int64)
    ed = np.asarray(edge_dst, np.int64)
    ew = np.asarray(edge_w, np.float32)

    # interleaved table id: shard r local row i -> (i//CS)*CT + r*CS + i%CS
    r = es // SR
    i = es % SR
    tid = (i // CS) * CT + r * CS + (i % CS)
    bucket = tid // BK
    loc = (tid % BK).astype(np.int16)

    core = ed // SR
    ldst = ed % SR
    dtile = ldst // P
    rel = (ldst % P).astype(np.float32)

    cell = ((core * NT + dtile) * NB + bucket).astype(np.int64)
    order = np.argsort(cell, kind="stable")
    cell_s = cell[order]
    loc_s = loc[order]
    rel_s = rel[order]
    w_s = ew[order]

    ncell = NCORES * NT * NB
    counts = np.bincount(cell_s, minlength=ncell)
    C = int(np.ceil(max(counts.max(), 128) / 128.0) * 128)
    C16 = C // 16
    KC = C // P

    starts = np.zeros(ncell + 1, np.int64)
    np.cumsum(counts, out=starts[1:])
    pos = np.arange(len(cell_s)) - starts[cell_s]  # position within cell

    # gather indices, wrapped [16, C16]: edge e -> [e%16, e//16]
    gidx = np.zeros((ncell, 16, C16), np.int16)
    gidx[cell_s, pos % 16, pos // 16] = loc_s
    gidx = np.tile(gidx.reshape(NCORES, NT, NB, 16, C16), (1, 1, 1, 8, 1))

    # dst-rel / weight, chunked [128, NB*KC]: edge e -> [e%128, b*KC + e//128]
    drel = np.full((ncell, P, KC), -1.0, np.float32)
    wv = np.zeros((ncell, P, KC), np.float32)
    drel[cell_s, pos % P, pos // P] = rel_s
    wv[cell_s, pos % P, pos // P] = w_s
    drel = drel.reshape(NCORES, NT, NB, P, KC).transpose(0, 1, 3, 2, 4) \
               .reshape(NCORES, NT, P, NB * KC).copy()
    wv = wv.reshape(NCORES, NT, NB, P, KC).transpose(0, 1, 3, 2, 4) \
           .reshape(NCORES, NT, P, NB * KC).copy()

    # x shards, transposed + blocked: [NT, 2, 128(feat), 128(row)]
    x = np.asarray(x, np.float32)
    xT = np.zeros((NCORES, NT, 2, P, P), np.float32)
    for c in range(NCORES):
        xs = np.zeros((SP, D), np.float32)
        xs[:SR] = x[c * SR:(c + 1) * SR]
        # [SP, 256] -> T [256, SP] -> [2, 128, NT, 128] -> [NT, 2, 128, 128]
        xT[c] = xs.T.reshape(2, P, NT, P).transpose(2, 0, 1, 3)

    return C, xT, gidx, drel, wv


def _build(C):
    """Build + compile the SPMD graph (shared by all 8 cores)."""
    C16 = C // 16
    KC = C // P
    NCOL = NB * KC
    nc = bacc.Bacc("TRN2", debug=False)

    dp = nc.declare_dram_parameter
    xT_p = dp("xT", [NT, 2, P, P], F32, isOutput=False)
    gidx_p = dp("gidx", [NT, NB, P, C16], I16, isOutput=False)
    drel_p = dp("drel", [NT, P, NCOL], F32, isOutput=False)
    wv_p = dp("wv", [NT, P, NCOL], F32, isOutput=False)
    gw_p = dp("gw", [NL, D, D], F32, isOutput=False)
    gb_p = dp("gb", [NL, 1, D], F32, isOutput=False)
    bngT_p = dp("bngT", [NL, P, 2], F32, isOutput=False)
    bnbT_p = dp("bnbT", [NL, P, 2], F32, isOutput=False)
    avgW_p = dp("avgW", [D, D], F32, isOutput=False)      # pre-scaled by 1/4
    avgb_p = dp("avgb", [1, D], F32, isOutput=False)
    lng_p = dp("lng", [1, D], F32, isOutput=False)
    lnb_p = dp("lnb", [1, D], F32, isOutput=False)
    lastW_p = dp("lastW", [D, NCLASS], F32, isOutput=False)
    lastb_p = dp("lastb", [1, NCLASS], F32, isOutput=False)
    ident_p = dp("ident", [P, P], F32, isOutput=False)
    pmask_p = dp("pmask", [P, 1], F32, isOutput=False)
    iota_p = dp("iota", [P, KC, P], F32, isOutput=False)
    out_p = dp("out", [SP, NCLASS], F32, isOutput=True)

    rg = [list(range(NCORES))]

    with tile.TileContext(nc) as tc:
        nc.gpsimd.load_library(library_config.mlp)
        with (
            tc.tile_pool(name="w", bufs=1) as wp,            # persistent weights
            tc.tile_pool(name="sc", bufs=1) as scp,          # bn scale/shift
            tc.tile_pool(name="sa", bufs=3) as sa,           # phase A working
            tc.tile_pool(name="st", bufs=2) as stp,          # stats working
            tc.tile_pool(name="gp", bufs=2) as gp,           # gather bufs
            tc.tile_pool(name="acc", bufs=1) as accp,        # 98 acc tiles
            tc.tile_pool(name="ps", bufs=2, space="PSUM") as psp,
            tc.tile_pool(name="psc", bufs=3, space="PSUM") as pscp,
            tc.tile_pool(name="dram", bufs=1, space="DRAM") as dr,
        ):
            # ---- persistent SBUF ----
            ident = wp.tile([P, P], F32, tag="ident")
            nc.sync.dma_start(ident[:], ident_p[:])
            iota = wp.tile([P, KC, P], F32, tag="iota")
            nc.sync.dma_start(iota[:], iota_p[:])
            ones = wp.tile([P, 1], F32, tag="ones")
            nc.vector.memset(ones[:], 1.0)
            eps_bn = wp.tile([P, 1], F32, tag="epsb")
            nc.vector.memset(eps_bn[:], BN_EPS)
            eps_ln = wp.tile([P, 1], F32, tag="epsl")
            nc.vector.memset(eps_ln[:], LN_EPS)
            slope_t = wp.tile([P, 1], F32, tag="slope")
            nc.vector.memset(slope_t[:], SLOPE)
            pmask = wp.tile([P, 1], F32, tag="pmask")
            nc.sync.dma_start(pmask[:], pmask_p[:])

            w_sb = []
            for L in range(NL):
                halves = []
                for h in range(2):
                    t_ = wp.tile([P, D], F32, tag=f"w{L}{h}")
                    nc.sync.dma_start(t_[:], gw_p[L, h * P:(h + 1) * P, :])
                    halves.append(t_)
                w_sb.append(halves)
            gb_bc = []
            for L in range(NL):
                t_ = wp.tile([P, D], F32, tag=f"gb{L}")
                nc.sync.dma_start(t_[:], gb_p[L].to_broadcast([P, D]))
                gb_bc.append(t_)
            bng_sb, bnb_sb = [], []
            for L in range(NL):
                tg = wp.tile([P, 2], F32, tag=f"bng{L}")
                nc.sync.dma_start(tg[:], bngT_p[L])
                bng_sb.append(tg)
                tb = wp.tile([P, 2], F32, tag=f"bnb{L}")
                nc.sync.dma_start(tb[:], bnbT_p[L])
                bnb_sb.append(tb)
            avgW_sb = []
            for h in range(2):
                t_ = wp.tile([P, D], F32, tag=f"avgW{h}")
                nc.sync.dma_start(t_[:], avgW_p[h * P:(h + 1) * P, :])
                avgW_sb.append(t_)
            avgb_bc = wp.tile([P, D], F32, tag="avgb")
            nc.sync.dma_start(avgb_bc[:], avgb_p[:].to_broadcast([P, D]))
            lng_bc = wp.tile([P, D], F32, tag="lng")
            nc.sync.dma_start(lng_bc[:], lng_p[:].to_broadcast([P, D]))
            lnb_bc = wp.tile([P, D], F32, tag="lnb")
            nc.sync.dma_start(lnb_bc[:], lnb_p[:].to_broadcast([P, D]))
            lastW_sb = []
            for h in range(2):
                t_ = wp.tile([P, NCLASS], F32, tag=f"lastW{h}")
                nc.sync.dma_start(t_[:], lastW_p[h * P:(h + 1) * P, :])
                lastW_sb.append(t_)
            lastb_bc = wp.tile([P, NCLASS], F32, tag="lastb")
            nc.sync.dma_start(lastb_bc[:], lastb_p[:].to_broadcast([P, NCLASS]))

            # ---- DRAM buffers ----
            table = [dr.tile([BK, D], F32, tag=f"tab{b}", name=f"tab{b}")
                     for b in range(NB)]
            sup_local = dr.tile([SP, D], F32, tag="sup")
            hraw = [dr.tile([SP, D], F32, tag=f"hraw{L}", name=f"hraw{L}")
                    for L in range(NL)]
            hnT = [dr.tile([NT, 2, P, P], F32, tag=f"hnT{L}", name=f"hnT{L}")
                   for L in range(3)]
            stats_in = dr.tile([P, 4], F32, tag="stin")
            stats_out = dr.tile([P, 4], F32, tag="stout")

            acc = [accp.tile([P, D], F32, tag=f"a{t}", name=f"a{t}")
                   for t in range(NT)]
            # bn scale/shift per layer/half, consumed next layer
            sc_sb = [[scp.tile([P, 1], F32, tag=f"sc{L}{h}", name=f"sc{L}{h}")
                      for h in range(2)]
                     for L in range(NL)]
            sh_sb = [[scp.tile([P, 1], F32, tag=f"sh{L}{h}", name=f"sh{L}{h}")
                      for h in range(2)]
                     for L in range(NL)]

            def phase_A(L):
                for t in range(NT):
                    hT = []
                    if L == 0:
                        for h in range(2):
                            ht = sa.tile([P, P], F32, tag=f"ht{h}")
                            nc.sync.dma_start(ht[:], xT_p[t, h])
                            hT.append(ht)
                    else:
                        hr = sa.tile([P, D], F32, tag="hr")
                        nc.sync.dma_start(hr[:], hraw[L - 1][t * P:(t + 1) * P, :])
                        for h in range(2):
                            pt = psp.tile([P, P], F32, tag="pt")
                            nc.tensor.transpose(
                                pt[:], hr[:, h * P:(h + 1) * P], ident[:])
                            zz = sa.tile([P, P], F32, tag=f"zz{h}")
                            nc.scalar.activation(
                                zz[:], pt[:], AF.Identity,
                                bias=sh_sb[L - 1][h][:, 0:1],
                                scale=sc_sb[L - 1][h][:, 0:1])
                            zt = sa.tile([P, P], F32, tag=f"zt{h}")
                            nc.vector.tensor_scalar(zt[:], zz[:], SLOPE, None,
                                                    op0=OP.mult)
                            ht = sa.tile([P, P], F32, tag=f"ht{h}")
                            nc.vector.tensor_tensor(out=ht[:], in0=zz[:],
                                                    in1=zt[:], op=OP.max)
                            nc.sync.dma_start(hnT[L - 1][t, h], ht[:])
                            hT.append(ht)
                    ps = psp.tile([P, D], F32, tag="mm")
                    for h in range(2):
                        nc.tensor.matmul(ps[:], lhsT=hT[h][:], rhs=w_sb[L][h][:],
                                         start=(h == 0), stop=(h == 1))
                    ssb = sa.tile([P, D], F32, tag="ssb")
                    nc.vector.tensor_copy(ssb[:], ps[:])
                    nc.sync.dma_start(sup_local[t * P:(t + 1) * P, :], ssb[:])

            def phase_B():
                for c in range(NCHUNK):
                    b, jj = divmod(c, NB)
                    nc.gpsimd.collective_compute(
                        "AllGather", OP.bypass, replica_groups=rg,
                        ins=[sup_local[c * CS:(c + 1) * CS, :].opt()],
                        outs=[table[b][jj * CT:(jj + 1) * CT, :].opt()])

            def phase_C(L):
                for b in range(NB):
                    for t in range(NT):
                        it = gp.tile([P, C16], I16, tag="idx")
                        nc.sync.dma_start(it[:], gidx_p[t, b])
                        g = gp.tile([P, KC, D], F32, tag="g")
                        # firmware limit: <= 1024 indices per dma_gather
                        nsplit = (C + 1023) // 1024
                        kper = (KC + nsplit - 1) // nsplit
                        for j0 in range(0, KC, kper):
                            j1 = min(KC, j0 + kper)
                            n = (j1 - j0) * P
                            nc.gpsimd.dma_gather(
                                g[:, j0:j1, :], table[b][:],
                                it[:, j0 * 8:j1 * 8],
                                num_idxs=n, num_idxs_reg=n, elem_size=D)
                        dw = gp.tile([P, 2 * KC], F32, tag="dw")
                        nc.sync.dma_start(dw[:, :KC],
                                          drel_p[t, :, b * KC:(b + 1) * KC])
                        nc.sync.dma_start(dw[:, KC:],
                                          wv_p[t, :, b * KC:(b + 1) * KC])
                        sel = gp.tile([P, KC, P], F32, tag="sel")
                        nc.vector.tensor_tensor(
                            out=sel[:], in0=dw[:, :KC].to_broadcast([P, KC, P]),
                            in1=iota[:], op=OP.is_equal)
                        nc.vector.tensor_tensor(
                            out=sel[:], in0=sel[:],
                            in1=dw[:, KC:].to_broadcast([P, KC, P]),
                            op=OP.mult)
                        pc = pscp.tile([P, D], F32, tag="pc")
                        for k in range(KC):
                            nc.tensor.matmul(pc[:], lhsT=sel[:, k, :],
                                             rhs=g[:, k, :],
                                             start=(k == 0), stop=(k == KC - 1))
                        if b == 0:
                            nc.vector.tensor_copy(acc[t][:], pc[:])
                        else:
                            nc.vector.tensor_add(acc[t][:], acc[t][:], pc[:])

            def phase_C2(L):
                ssum = stp.tile([P, D], F32, tag="ssum")
                ssq = stp.tile([P, D], F32, tag="ssq")
                for t in range(NT):
                    hs = sa.tile([P, D], F32, tag="hs")
                    nc.vector.tensor_add(hs[:], acc[t][:], gb_bc[L][:])
                    if t == NT - 1:
                        nc.vector.tensor_scalar(hs[:], hs[:], pmask[:, 0:1],
                                                None, op0=OP.mult)
                    nc.sync.dma_start(hraw[L][t * P:(t + 1) * P, :], hs[:])
                    sq = sa.tile([P, D], F32, tag="sq")
                    nc.vector.tensor_tensor(out=sq[:], in0=hs[:], in1=hs[:],
                                            op=OP.mult)
                    if t == 0:
                        nc.vector.tensor_copy(ssum[:], hs[:])
                        nc.vector.tensor_copy(ssq[:], sq[:])
                    else:
                        nc.vector.tensor_add(ssum[:], ssum[:], hs[:])
                        nc.vector.tensor_add(ssq[:], ssq[:], sq[:])
                return ssum, ssq

            def phase_D(L, ssum, ssq):
                pstat = psp.tile([P, D], F32, tag="mm")
                for h in range(2):
                    nc.tensor.matmul(pstat[:, h:h + 1],
                                     lhsT=ssum[:, h * P:(h + 1) * P],
                                     rhs=ones[:], start=True, stop=True)
                    nc.tensor.matmul(pstat[:, 2 + h:3 + h],
                                     lhsT=ssq[:, h * P:(h + 1) * P],
                                     rhs=ones[:], start=True, stop=True)
                sstat = stp.tile([P, 4], F32, tag="sstat")
                nc.vector.tensor_copy(sstat[:], pstat[:, 0:4])
                nc.sync.dma_start(stats_in[:], sstat[:])
                nc.gpsimd.collective_compute(
                    "AllReduce", OP.add, replica_groups=rg,
                    ins=[stats_in[:].opt()], outs=[stats_out[:].opt()])
                st = stp.tile([P, 4], F32, tag="strd")
                nc.sync.dma_start(st[:], stats_out[:])
                inv_n = 1.0 / float(N)
                for h in range(2):
                    mean = stp.tile([P, 1], F32, tag="mean")
                    nc.vector.tensor_scalar(mean[:], st[:, h:h + 1], inv_n, None,
                                            op0=OP.mult)
                    msq = stp.tile([P, 1], F32, tag="msq")
                    nc.vector.tensor_scalar(msq[:], st[:, 2 + h:3 + h], inv_n,
                                            None, op0=OP.mult)
                    m2 = stp.tile([P, 1], F32, tag="m2")
                    nc.vector.tensor_tensor(out=m2[:], in0=mean[:], in1=mean[:],
                                            op=OP.mult)
                    var = stp.tile([P, 1], F32, tag="var")
                    nc.vector.tensor_tensor(out=var[:], in0=msq[:], in1=m2[:],
                                            op=OP.subtract)
                    sd = stp.tile([P, 1], F32, tag="sd")
                    nc.scalar.activation(sd[:], var[:], AF.Sqrt, bias=eps_bn[:, 0:1],
                                         scale=1.0)
                    rstd = stp.tile([P, 1], F32, tag="rstd")
                    nc.vector.reciprocal(rstd[:], sd[:])
                    nc.vector.tensor_tensor(out=sc_sb[L][h][:],
                                            in0=bng_sb[L][:, h:h + 1],
                                            in1=rstd[:], op=OP.mult)
                    t1 = stp.tile([P, 1], F32, tag="t1")
                    nc.vector.tensor_tensor(out=t1[:], in0=mean[:],
                                            in1=sc_sb[L][h][:], op=OP.mult)
                    nc.vector.tensor_tensor(out=sh_sb[L][h][:],
                                            in0=bnb_sb[L][:, h:h + 1],
                                            in1=t1[:], op=OP.subtract)

            def phase_E():
                L = NL - 1
                for t in range(NT):
                    hr = sa.tile([P, D], F32, tag="hr")
                    nc.sync.dma_start(hr[:], hraw[L][t * P:(t + 1) * P, :])
                    pile = []
                    h3nT = []
                    for h in range(2):
                        pt = psp.tile([P, P], F32, tag="pt")
                        nc.tensor.transpose(pt[:], hr[:, h * P:(h + 1) * P],
                                            ident[:])
                        zz = sa.tile([P, P], F32, tag=f"zz{h}")
                        nc.scalar.activation(zz[:], pt[:], AF.Identity,
                                             bias=sh_sb[L][h][:, 0:1],
                                             scale=sc_sb[L][h][:, 0:1])
                        zt = sa.tile([P, P], F32, tag=f"zt{h}")
                        nc.vector.tensor_scalar(zt[:], zz[:], SLOPE, None,
                                                op0=OP.mult)
                        ht = sa.tile([P, P], F32, tag=f"ht{h}")
                        nc.vector.tensor_tensor(out=ht[:], in0=zz[:],
                                                in1=zt[:], op=OP.max)
                        h3nT.append(ht)
                        pl = sa.tile([P, P], F32, tag=f"pl{h}")
                        p0 = sa.tile([P, P], F32, tag=f"p0{h}")
                        nc.sync.dma_start(p0[:], hnT[0][t, h])
                        nc.vector.tensor_add(pl[:], p0[:], ht[:])
                        for Lp in (1, 2):
                            px = sa.tile([P, P], F32, tag=f"px{h}")
                            nc.sync.dma_start(px[:], hnT[Lp][t, h])
                            nc.vector.tensor_add(pl[:], pl[:], px[:])
                        pile.append(pl)
                    pa = psp.tile([P, D], F32, tag="mm")
                    for h in range(2):
                        nc.tensor.matmul(pa[:], lhsT=pile[h][:],
                                         rhs=avgW_sb[h][:],
                                         start=(h == 0), stop=(h == 1))
                    asb = sa.tile([P, D], F32, tag="asb")
                    nc.vector.tensor_add(asb[:], pa[:], avgb_bc[:])
                    # LayerNorm along features (free dim)
                    mu = sa.tile([P, 1], F32, tag="mu")
                    nc.vector.reduce_sum(mu[:], asb[:], axis=AX.X)
                    nc.vector.tensor_scalar(mu[:], mu[:], 1.0 / D, None,
                                            op0=OP.mult)
                    ctr = sa.tile([P, D], F32, tag="ctr")
                    nc.vector.tensor_scalar(ctr[:], asb[:], mu[:, 0:1], None,
                                            op0=OP.subtract)
                    sq = sa.tile([P, D], F32, tag="sqE")
                    nc.vector.tensor_tensor(out=sq[:], in0=ctr[:], in1=ctr[:],
                                            op=OP.mult)
                    var = sa.tile([P, 1], F32, tag="varE")
                    nc.vector.reduce_sum(var[:], sq[:], axis=AX.X)
                    nc.vector.tensor_scalar(var[:], var[:], 1.0 / D, None,
                                            op0=OP.mult)
                    sd = sa.tile([P, 1], F32, tag="sdE")
                    nc.scalar.activation(sd[:], var[:], AF.Sqrt, bias=eps_ln[:, 0:1],
                                         scale=1.0)
                    rstd = sa.tile([P, 1], F32, tag="rstdE")
                    nc.vector.reciprocal(rstd[:], sd[:])
                    lnv = sa.tile([P, D], F32, tag="lnv")
                    nc.vector.tensor_scalar(lnv[:], ctr[:], rstd[:, 0:1], None,
                                            op0=OP.mult)
                    nc.vector.tensor_tensor(out=lnv[:], in0=lnv[:], in1=lng_bc[:],
                                            op=OP.mult)
                    nc.vector.tensor_tensor(out=lnv[:], in0=lnv[:], in1=lnb_bc[:],
                                            op=OP.add)
                    # h3n row-major (transpose back), h_final = .8*h3n + .2*ln
                    hf = sa.tile([P, D], F32, tag="hf")
                    for h in range(2):
                        pt2 = psp.tile([P, P], F32, tag="pt")
                        nc.tensor.transpose(pt2[:], h3nT[h][:], ident[:])
                        nc.vector.tensor_scalar(hf[:, h * P:(h + 1) * P],
                                                pt2[:], 1.0 - ALPHA, None,
                                                op0=OP.mult)
                    nc.vector.tensor_scalar(lnv[:], lnv[:], ALPHA, None,
                                            op0=OP.mult)
                    nc.vector.tensor_add(hf[:], hf[:], lnv[:])
                    # logits = hf @ lastW + lastb
                    hfT = []
                    for h in range(2):
                        pt3 = psp.tile([P, P], F32, tag="pt")
                        nc.tensor.transpose(pt3[:], hf[:, h * P:(h + 1) * P],
                                            ident[:])
                        htf = sa.tile([P, P], F32, tag=f"htf{h}")
                        nc.vector.tensor_copy(htf[:], pt3[:])
                        hfT.append(htf)
                    pl2 = psp.tile([P, D], F32, tag="mm")
                    for h in range(2):
                        nc.tensor.matmul(pl2[:, 0:NCLASS], lhsT=hfT[h][:],
                                         rhs=lastW_sb[h][:],
                                         start=(h == 0), stop=(h == 1))
                    lg = sa.tile([P, NCLASS], F32, tag="lg")
                    nc.vector.tensor_add(lg[:], pl2[:, 0:NCLASS], lastb_bc[:])
                    mx = sa.tile([P, 1], F32, tag="mx")
                    nc.vector.reduce_max(mx[:], lg[:], axis=AX.X)
                    z = sa.tile([P, NCLASS], F32, tag="z")
                    nc.vector.tensor_scalar(z[:], lg[:], mx[:, 0:1], None,
                                            op0=OP.subtract)
                    e = sa.tile([P, NCLASS], F32, tag="e")
                    nc.scalar.activation(e[:], z[:], AF.Exp)
                    s = sa.tile([P, 1], F32, tag="s")
                    nc.vector.reduce_sum(s[:], e[:], axis=AX.X)
                    ls = sa.tile([P, 1], F32, tag="ls")
                    nc.scalar.activation(ls[:], s[:], AF.Ln)
                    o = sa.tile([P, NCLASS], F32, tag="o")
                    nc.vector.tensor_scalar(o[:], z[:], ls[:, 0:1], None,
                                            op0=OP.subtract)
                    nc.sync.dma_start(out_p[t * P:(t + 1) * P, :], o[:])

            nlayers = int(os.environ.get("BGNN_LAYERS", str(NL)))
            ph = os.environ.get("BGNN_PH", "ABC2DE")
            for L in range(nlayers):
                if "A" in ph:
                    phase_A(L)
                if "B" in ph:
                    phase_B()
                if "C" in ph:
                    phase_C(L)
                if "2" in ph:
                    ssum, ssq = phase_C2(L)
                if "D" in ph:
                    phase_D(L, ssum, ssq)
            if "E" in ph and nlayers == NL:
                phase_E()
            else:
                # debug fallback: make sure `out` is written
                for t in range(NT):
                    o = sa.tile([P, NCLASS], F32, tag="o")
                    nc.vector.tensor_copy(o[:], gb_bc[0][:, 0:NCLASS])
                    nc.sync.dma_start(out_p[t * P:(t + 1) * P, :], o[:])

    nc.compile()
    return nc


def kernel(x, edge_src, edge_dst, edge_w, gnn_W, gnn_b, bn_gamma, bn_beta,
           avg_W, avg_b, ln_g, ln_b, last_W, last_b):
    C, xT, gidx, drel, wv = _preprocess(x, edge_src, edge_dst, edge_w)

    if C not in _CACHE:
        _CACHE[C] = _build(C)
    nc = _CACHE[C]

    gw = np.ascontiguousarray(np.asarray(gnn_W, np.float32))
    gb = np.asarray(gnn_b, np.float32).reshape(NL, 1, D)
    bngT = np.ascontiguousarray(
        np.asarray(bn_gamma, np.float32).reshape(NL, 2, P).transpose(0, 2, 1))
    bnbT = np.ascontiguousarray(
        np.asarray(bn_beta, np.float32).reshape(NL, 2, P).transpose(0, 2, 1))
    avgW = np.ascontiguousarray(np.asarray(avg_W, np.float32) * (1.0 / NL))
    avgb = np.asarray(avg_b, np.float32).reshape(1, D)
    lng = np.asarray(ln_g, np.float32).reshape(1, D)
    lnb = np.asarray(ln_b, np.float32).reshape(1, D)
    lastW = np.ascontiguousarray(np.asarray(last_W, np.float32))
    lastb = np.asarray(last_b, np.float32).reshape(1, NCLASS)
    ident = np.eye(P, dtype=np.float32)
    pmask = (np.arange(P) < (SR - (NT - 1) * P)).astype(np.float32).reshape(P, 1)
    KC = C // P
    iota = np.ascontiguousarray(
        np.tile(np.arange(P, dtype=np.float32)[None, None, :], (P, KC, 1)))

    shared = dict(gw=gw, gb=gb, bngT=bngT, bnbT=bnbT, avgW=avgW, avgb=avgb,
                  lng=lng, lnb=lnb, lastW=lastW, lastb=lastb, ident=ident,
                  iota=iota, pmask=pmask)
    in_maps = []
    for c in range(NCORES):
        m = dict(shared)
        m["xT"] = np.ascontiguousarray(xT[c])
        m["gidx"] = np.ascontiguousarray(gidx[c])
        m["drel"] = np.ascontiguousarray(drel[c])
        m["wv"] = np.ascontiguousarray(wv[c])
        in_maps.append(m)

    trace = bool(int(os.environ.get("BASS_GNN_TRACE", "0")))
    res = run_bass_kernel_spmd(nc, in_maps, core_ids=list(range(NCORES)),
                               trace=trace)
    global LAST_EXEC_NS, LAST_RESULT
    LAST_EXEC_NS = getattr(res, "exec_time_ns", None)
    LAST_RESULT = res
    out = np.concatenate(
        [res.results[c]["out"][:SR] for c in range(NCORES)], axis=0)
    return out.astype(np.float32)


LAST_EXEC_NS = None
LAST_RESULT = None

